# revision 11
# baseline (speedup 1.0000x reference)
"""GQA attention (B=2,T=2048,C=2048,NH=16,NKV=4,HD=128) + RoPE + causal,
sharded over 8 NeuronCores as (batch, kv-group); Bass/Tile kernel.

Each core (b, g) computes, for batch b and KV group g (4 Q heads):
  Qt_h = (x_b @ Wq_h)^T          [HD=128, T]   (RoPE applied)
  Kt   = (x_b @ Wk_g)^T          [128, T]      (RoPE applied)
  V    = x_b @ Wv_g              [T, 128]      (via Vt + PE transpose)
  St   = Kt^T-tiles . Qt         [k, q] score tiles (transposed scores)
  Pt   = exp(St/sqrt(HD)) * causal_mask        (no max-shift: logits are O(5))
  Ot_h = V^T-tiles . Pt          [HD, q] unnormalized
  d    = ones . Pacc             softmax denominators per q (ones-matmul)
  Otn  = Ot * (1/d broadcast)    (K=1 outer-product matmul for the bcast)
  yt  += Wo_g-slice^T . Otn      [C, T] partial output, transposed

Host: shards inputs, provides x^T and RoPE tables; output y[b] = (sum_g yt)^T.
"""

import re
import sys

import numpy as np

if "/opt/trn_rl_repo" not in sys.path:
    sys.path.insert(0, "/opt/trn_rl_repo")

import concourse.bass as bass
import concourse.mybir as mybir
import concourse.tile as tile
from concourse.bass_utils import run_bass_kernel_spmd
from concourse.masks import make_identity
from concourse.vector_clock import ScopedClock, VectorClock

B, T, C = 2, 2048, 2048
NH, NKV = 16, 4
HD = C // NH            # 128
GH = NH // NKV          # 4 heads per kv group
ROPE_THETA = 10000.0
SCALE = 1.0 / float(np.sqrt(HD))
NT = T // 128           # 16 t-tiles of 128
NTB = T // 512          # 4 t-blocks of 512
NCT = C // 128          # 16 c-tiles
F32 = mybir.dt.float32
F32R = mybir.dt.float32r
PV_PIPE = 3             # St runs this many kt-tiles ahead of PV


def _patch_tile_drain():
    """walrus in this container rejects CTRL instructions with >1 sync wait;
    split the TileContext tail drain into one drain per outstanding proc."""
    if getattr(tile.TileContext, "_drain_patched", False):
        return

    def _drain_and_barrier(self, tick_clock, wait_clock):
        gc = tick_clock.global_clock
        vals = [int(s) for s in re.findall(r"\d+", repr(gc))]
        for idx, val in [(i, v) for i, v in enumerate(vals) if v > 0]:
            drain_inst = self.nc.sync.drain()
            sub = VectorClock()
            sub.require_at_least(idx, val)
            wait_clock.add_sem_waits(drain_inst.ins, ScopedClock({None: sub}))
        self.nc.all_engine_barrier()
        popped = self.nc._tile_sem_poison_stack.pop()
        assert popped is self._sem_poison
        self.nc.clear_and_free_semaphores(list(self.sems.allocated().values()))
        self.nc.all_engine_barrier()

    tile.TileContext._drain_and_barrier = _drain_and_barrier
    tile.TileContext._drain_patched = True


def _split_multi_waits(nc, max_waits=1):
    """This container's walrus rejects instructions carrying more than one
    sync wait: hoist excess waits onto same-engine NOPs inserted before."""
    n = 0
    for f in nc.m.functions:
        for blk in f.blocks:
            il = blk.instructions
            i = 0
            while i < len(il):
                ins = il[i]
                si = ins.sync_info
                if si is not None and len(si.on_wait) > max_waits:
                    waits = list(si.on_wait)
                    extra = waits[:-max_waits]
                    for w in extra:
                        nop = mybir.InstNoOp(name=f"wsplit_{n}", ins=[], outs=[])
                        n += 1
                        nop.engine = ins.engine
                        nop.sync_info = type(si)(on_wait=[w], on_update=[])
                        il.insert(i, nop)
                        i += 1
                    ins.sync_info = type(si)(
                        on_wait=waits[-max_waits:], on_update=list(si.on_update))
                i += 1
            assert len(blk.instructions) == len(il)


def build_kernel():
    _patch_tile_drain()
    nc = bass.Bass("TRN2", target_bir_lowering=False, debug=False)

    xT = nc.dram_tensor("xT", [C, T], F32R, kind="ExternalInput")
    wq = nc.dram_tensor("wq", [C, GH * HD], F32R, kind="ExternalInput")
    wk = nc.dram_tensor("wk", [C, HD], F32R, kind="ExternalInput")
    wv = nc.dram_tensor("wv", [C, HD], F32R, kind="ExternalInput")
    wo = nc.dram_tensor("wo", [GH * HD, C], F32R, kind="ExternalInput")
    cosT = nc.dram_tensor("cosT", [HD, T], F32, kind="ExternalInput")
    sinT = nc.dram_tensor("sinT", [HD, T], F32, kind="ExternalInput")
    yt = nc.dram_tensor("yt", [C, T], F32, kind="ExternalOutput")

    with tile.TileContext(nc) as tc:
        with (
            tc.tile_pool(name="consts", bufs=1) as consts,
            tc.tile_pool(name="wsmall", bufs=1) as wsmall,
            tc.tile_pool(name="wbig", bufs=1) as wbig,       # Wq then Wo (shared slots)
            tc.tile_pool(name="big8k", bufs=6) as big8k,     # cos,sin then 4x Ot
            tc.tile_pool(name="qk", bufs=1) as qkpool,
            tc.tile_pool(name="xs", bufs=4) as xs,
            tc.tile_pool(name="rope", bufs=3) as ropep,
            tc.tile_pool(name="ptp", bufs=6) as ptp,
            tc.tile_pool(name="pacc", bufs=2) as paccp,
            tc.tile_pool(name="rdp", bufs=2) as rdp,
            tc.tile_pool(name="yo", bufs=3) as yop,
        ):
            # ---- constants (built in f32, converted to f32r via DVE copy) ----
            mbig32 = consts.tile([128, 896], F32)
            nc.gpsimd.memset(mbig32, 1.0)
            nc.gpsimd.affine_select(
                out=mbig32, in_=mbig32,
                compare_op=mybir.AluOpType.is_ge,
                fill=0.0, base=-384,
                pattern=[[1, 896]], channel_multiplier=-1,
            )
            mbig = consts.tile([128, 896], F32R)      # shifted causal masks
            nc.vector.tensor_copy(out=mbig, in_=mbig32)
            ident32 = consts.tile([128, 128], F32)
            make_identity(nc, ident32)
            ident = consts.tile([128, 128], F32R)
            nc.vector.tensor_copy(out=ident, in_=ident32)
            ones32 = consts.tile([128, 1], F32)
            nc.vector.memset(ones32, 1.0)
            ones128 = consts.tile([128, 1], F32R)     # densum lhsT  [K=128, M=1]
            nc.vector.tensor_copy(out=ones128, in_=ones32)
            onesr32 = consts.tile([1, 128], F32)
            nc.vector.memset(onesr32, 1.0)
            ones_row = consts.tile([1, 128], F32R)    # bcast lhsT   [K=1, M=128]
            nc.vector.tensor_copy(out=ones_row, in_=onesr32)

            # ---- resident weights / tables ----
            wq_sb = wbig.tile([128, NCT, GH * HD], F32R, tag="wbig")
            nc.sync.dma_start(out=wq_sb, in_=wq.rearrange("(ct p) n -> p ct n", p=128))
            wk_sb = wsmall.tile([128, NCT, HD], F32R, tag="wk")
            nc.sync.dma_start(out=wk_sb, in_=wk.rearrange("(ct p) n -> p ct n", p=128))
            wv_sb = wsmall.tile([128, NCT, HD], F32R, tag="wv")
            nc.sync.dma_start(out=wv_sb, in_=wv.rearrange("(ct p) n -> p ct n", p=128))
            cos_sb = big8k.tile([128, T], F32, tag="big8k")
            nc.sync.dma_start(out=cos_sb, in_=cosT[:, :])
            sin_sb = big8k.tile([128, T], F32, tag="big8k")
            nc.sync.dma_start(out=sin_sb, in_=sinT[:, :])

            qt_sb = [qkpool.tile([128, T], F32R, tag=f"qt{h}", name=f"qt{h}")
                     for h in range(GH)]
            kt_sb = qkpool.tile([128, T], F32R, tag="kt")
            v_sb = qkpool.tile([128, NT, HD], F32R, tag="v")

            # ================= phase 1: projections =================
            def rope_store(ps, dest, tb):
                """dest[:, tb*512:(tb+1)*512] = rope(ps) ; ps is [128(d), 512(t)]"""
                sl = slice(tb * 512, (tb + 1) * 512)
                a = ropep.tile([128, 512], F32, tag="ropea")
                nc.vector.tensor_mul(a, ps, cos_sb[:, sl])
                b = ropep.tile([128, 512], F32, tag="ropeb")
                nc.vector.tensor_mul(b[0:64], ps[64:128], sin_sb[0:64, sl])
                nc.vector.tensor_mul(b[64:128], ps[0:64], sin_sb[64:128, sl])
                nc.vector.tensor_sub(dest[0:64, sl], a[0:64], b[0:64])
                nc.vector.tensor_add(dest[64:128, sl], a[64:128], b[64:128])

            with (
                tc.tile_pool(name="pp", bufs=6, space="PSUM") as pp,
                tc.tile_pool(name="pvt", bufs=2, space="PSUM") as pvt,
                tc.tile_pool(name="vtt", bufs=2) as vtt,
            ):
                for tb in range(NTB):
                    ps_q = [pp.tile([128, 512], F32, tag="pp", name=f"psq{h}")
                            for h in range(GH)]
                    ps_k = pp.tile([128, 512], F32, tag="pp")
                    ps_v = pp.tile([128, 512], F32, tag="pp")
                    for ct in range(NCT):
                        xt = xs.tile([128, 512], F32R, tag="xs")
                        nc.sync.dma_start(
                            out=xt,
                            in_=xT[ct * 128:(ct + 1) * 128, tb * 512:(tb + 1) * 512],
                        )
                        st, sp = (ct == 0), (ct == NCT - 1)
                        for h in range(GH):
                            nc.tensor.matmul(
                                ps_q[h], (wq_sb[:, ct, h * HD:(h + 1) * HD]),
                                (xt), start=st, stop=sp,
                            )
                        nc.tensor.matmul(
                            ps_k, (wk_sb[:, ct, :]), (xt), start=st, stop=sp)
                        nc.tensor.matmul(
                            ps_v, (wv_sb[:, ct, :]), (xt), start=st, stop=sp)
                    for h in range(GH):
                        rope_store(ps_q[h], qt_sb[h], tb)
                    rope_store(ps_k, kt_sb, tb)
                    # V: copy Vt block to SBUF, PE-transpose each 128x128 tile
                    vt = vtt.tile([128, 512], F32R, tag="vtt")
                    nc.scalar.copy(out=vt, in_=ps_v)
                    for j in range(4):
                        ps_t = pvt.tile([128, 128], F32R, tag="pvt")
                        with nc.allow_low_precision(reason="fp32r PE transpose of V"):
                            nc.tensor.transpose(
                                ps_t, vt[:, j * 128:(j + 1) * 128], ident)
                        nc.scalar.copy(out=v_sb[:, tb * 4 + j, :], in_=ps_t)

            # ================= phase 2: attention =================
            wo_sb = wbig.tile([128, GH, C], F32R, tag="wbig")
            nc.sync.dma_start(out=wo_sb, in_=wo.rearrange("(h p) c -> p h c", p=128))
            ot_sb = [big8k.tile([128, T], F32R, tag="big8k", name=f"ot{h}")
                     for h in range(GH)]

            with (
                tc.tile_pool(name="pst", bufs=4, space="PSUM") as pst,
                tc.tile_pool(name="pot", bufs=2, space="PSUM") as pot,
                tc.tile_pool(name="pd", bufs=1, space="PSUM") as pd,
                tc.tile_pool(name="prdb", bufs=1, space="PSUM") as prdb,
            ):
                for h in range(GH):
                    for qb in range(NTB):
                        nkt = 4 * qb + 4
                        qsl = slice(qb * 512, (qb + 1) * 512)
                        ps_ot = pot.tile([128, 512], F32, tag="pot")
                        pacc = paccp.tile([128, 512], F32R, tag="pacc")
                        pts = [None] * nkt
                        ps_d = None

                        def emit_st(kt):
                            ps_st = pst.tile([128, 512], F32, tag="pst")
                            nc.tensor.matmul(
                                ps_st, (kt_sb[:, kt * 128:(kt + 1) * 128]),
                                (qt_sb[h][:, qsl]), start=True, stop=True,
                            )
                            pt = ptp.tile([128, 512], F32R, tag="pt")
                            nc.scalar.activation(
                                out=pt, in_=ps_st,
                                func=mybir.ActivationFunctionType.Exp, scale=SCALE,
                            )
                            if kt >= 4 * qb:  # diagonal block: causal mask
                                i = kt - 4 * qb
                                nc.vector.tensor_mul(
                                    pt, pt, mbig[:, 384 - 128 * i: 896 - 128 * i])
                            if kt == 0:
                                nc.vector.tensor_copy(out=pacc, in_=pt)
                            else:
                                nc.vector.tensor_add(pacc, pacc, pt)
                            pts[kt] = pt

                        def emit_pv(kt):
                            nc.tensor.matmul(
                                ps_ot, (v_sb[:, kt, :]), (pts[kt]),
                                start=(kt == 0), stop=(kt == nkt - 1),
                            )

                        for kt in range(nkt):
                            emit_st(kt)
                            if kt == nkt - 1:  # denominators (only needs pacc)
                                ps_d = pd.tile([1, 512], F32, tag="pd")
                                nc.tensor.matmul(
                                    ps_d, (ones128), (pacc),
                                    start=True, stop=True)
                            if kt >= PV_PIPE:
                                emit_pv(kt - PV_PIPE)
                        for kt in range(max(0, nkt - PV_PIPE), nkt):
                            emit_pv(kt)

                        rd = rdp.tile([1, 512], F32R, tag="rd")
                        with nc.allow_low_precision(reason="softmax denom recip to fp32r"):
                            nc.vector.reciprocal(out=rd, in_=ps_d)
                        ps_rdb = prdb.tile([128, 512], F32, tag="prdb")
                        nc.tensor.matmul(
                            ps_rdb, (ones_row), (rd), start=True, stop=True)
                        rdb_sb = ropep.tile([128, 512], F32, tag="ropea",
                                            name=f"rdb{h}_{qb}")
                        nc.scalar.copy(out=rdb_sb, in_=ps_rdb)
                        nc.vector.tensor_mul(ot_sb[h][:, qsl], ps_ot, rdb_sb)

            # ================= phase 3: output projection =================
            with tc.tile_pool(name="py", bufs=4, space="PSUM") as py:
                for ct in range(NCT):
                    for tb in range(NTB):
                        ps_y = py.tile([128, 512], F32, tag="py")
                        for h in range(GH):
                            nc.tensor.matmul(
                                ps_y, (wo_sb[:, h, ct * 128:(ct + 1) * 128]),
                                (ot_sb[h][:, tb * 512:(tb + 1) * 512]),
                                start=(h == 0), stop=(h == GH - 1),
                            )
                        yo = yop.tile([128, 512], F32, tag="yo")
                        nc.vector.tensor_copy(out=yo, in_=ps_y)
                        nc.sync.dma_start(
                            out=yt[ct * 128:(ct + 1) * 128, tb * 512:(tb + 1) * 512],
                            in_=yo,
                        )
    _split_multi_waits(nc)
    return nc


def _rope_tables():
    inv_freq = 1.0 / (ROPE_THETA ** (np.arange(0, HD, 2, dtype=np.float32) / HD))
    t = np.arange(T, dtype=np.float32)
    freqs = np.outer(t, inv_freq)                    # [T, HD/2]
    emb = np.concatenate([freqs, freqs], axis=-1)    # [T, HD]
    cosT = np.ascontiguousarray(np.cos(emb).T.astype(np.float32))  # [HD, T]
    sinT = np.ascontiguousarray(np.sin(emb).T.astype(np.float32))
    return cosT, sinT


_NC_CACHE = {}


def kernel(x, Wq, Wk, Wv, Wo, _trace=False, _trace_kwargs=None):
    x = np.asarray(x, np.float32)
    Wq, Wk, Wv, Wo = (np.asarray(w, np.float32) for w in (Wq, Wk, Wv, Wo))
    if "nc" not in _NC_CACHE:
        _NC_CACHE["nc"] = build_kernel()
    nc = _NC_CACHE["nc"]

    cosT, sinT = _rope_tables()
    in_maps = []
    for b in range(B):
        xTb = np.ascontiguousarray(x[b].T)
        for g in range(NKV):
            in_maps.append({
                "xT": xTb,
                "wq": np.ascontiguousarray(Wq[:, g * 512:(g + 1) * 512]),
                "wk": np.ascontiguousarray(Wk[:, g * 128:(g + 1) * 128]),
                "wv": np.ascontiguousarray(Wv[:, g * 128:(g + 1) * 128]),
                "wo": np.ascontiguousarray(Wo[g * 512:(g + 1) * 512, :]),
                "cosT": cosT,
                "sinT": sinT,
            })

    kwargs = {}
    if _trace:
        kwargs["trace"] = True
        kwargs.update(_trace_kwargs or {})
    res = run_bass_kernel_spmd(nc, in_maps, core_ids=list(range(8)), **kwargs)

    y = np.empty((B, T, C), np.float32)
    for b in range(B):
        acc = res.results[b * NKV]["yt"].astype(np.float32)
        for g in range(1, NKV):
            acc = acc + res.results[b * NKV + g]["yt"]
        y[b] = acc.T
    if _trace:
        return y, res
    return y


# revision 13
# speedup vs baseline: 1.0235x; 1.0235x over previous
"""GQA attention (B=2,T=2048,C=2048,NH=16,NKV=4,HD=128) + RoPE + causal,
sharded over 8 NeuronCores as (batch, kv-group); Bass/Tile kernel.

Each core (b, g) computes, for batch b and KV group g (4 Q heads):
  Qt_h = (x_b @ Wq_h)^T          [HD=128, T]   (RoPE applied)
  Kt   = (x_b @ Wk_g)^T          [128, T]      (RoPE applied)
  V    = x_b @ Wv_g              [T, 128]      (via Vt + PE transpose)
  St   = Kt^T-tiles . Qt         [k, q] score tiles (transposed scores)
  Pt   = exp(St/sqrt(HD)) * causal_mask        (no max-shift: logits are O(5))
  Ot_h = V^T-tiles . Pt          [HD, q] unnormalized
  d    = ones . Pacc             softmax denominators per q (ones-matmul)
  Otn  = Ot * (1/d broadcast)    (K=1 outer-product matmul for the bcast)
  yt  += Wo_g-slice^T . Otn      [C, T] partial output, transposed

Host: shards inputs, provides x^T and RoPE tables; output y[b] = (sum_g yt)^T.
"""

import re
import sys

import numpy as np

if "/opt/trn_rl_repo" not in sys.path:
    sys.path.insert(0, "/opt/trn_rl_repo")

import concourse.bass as bass
import concourse.mybir as mybir
import concourse.tile as tile
from concourse.bass_utils import run_bass_kernel_spmd
from concourse.masks import make_identity
from concourse.vector_clock import ScopedClock, VectorClock

B, T, C = 2, 2048, 2048
NH, NKV = 16, 4
HD = C // NH            # 128
GH = NH // NKV          # 4 heads per kv group
ROPE_THETA = 10000.0
SCALE = 1.0 / float(np.sqrt(HD))
NT = T // 128           # 16 t-tiles of 128
NTB = T // 512          # 4 t-blocks of 512
NCT = C // 128          # 16 c-tiles
F32 = mybir.dt.float32
F32R = mybir.dt.float32r
PV_PIPE = 3             # St runs this many kt-tiles ahead of PV


def _patch_tile_drain():
    """walrus in this container rejects CTRL instructions with >1 sync wait;
    split the TileContext tail drain into one drain per outstanding proc."""
    if getattr(tile.TileContext, "_drain_patched", False):
        return

    def _drain_and_barrier(self, tick_clock, wait_clock):
        gc = tick_clock.global_clock
        vals = [int(s) for s in re.findall(r"\d+", repr(gc))]
        for idx, val in [(i, v) for i, v in enumerate(vals) if v > 0]:
            drain_inst = self.nc.sync.drain()
            sub = VectorClock()
            sub.require_at_least(idx, val)
            wait_clock.add_sem_waits(drain_inst.ins, ScopedClock({None: sub}))
        self.nc.all_engine_barrier()
        popped = self.nc._tile_sem_poison_stack.pop()
        assert popped is self._sem_poison
        self.nc.clear_and_free_semaphores(list(self.sems.allocated().values()))
        self.nc.all_engine_barrier()

    tile.TileContext._drain_and_barrier = _drain_and_barrier
    tile.TileContext._drain_patched = True


def _split_multi_waits(nc, max_waits=1):
    """This container's walrus rejects instructions carrying more than one
    sync wait: hoist excess waits onto same-engine NOPs inserted before."""
    n = 0
    for f in nc.m.functions:
        for blk in f.blocks:
            il = blk.instructions
            i = 0
            while i < len(il):
                ins = il[i]
                si = ins.sync_info
                if si is not None and len(si.on_wait) > max_waits:
                    waits = list(si.on_wait)
                    extra = waits[:-max_waits]
                    for w in extra:
                        nop = mybir.InstNoOp(name=f"wsplit_{n}", ins=[], outs=[])
                        n += 1
                        nop.engine = ins.engine
                        nop.sync_info = type(si)(on_wait=[w], on_update=[])
                        il.insert(i, nop)
                        i += 1
                    ins.sync_info = type(si)(
                        on_wait=waits[-max_waits:], on_update=list(si.on_update))
                i += 1
            assert len(blk.instructions) == len(il)


def build_kernel():
    _patch_tile_drain()
    nc = bass.Bass("TRN2", target_bir_lowering=False, debug=False)

    xT = nc.dram_tensor("xT", [C, T], F32R, kind="ExternalInput")
    wq = nc.dram_tensor("wq", [C, GH * HD], F32R, kind="ExternalInput")
    wk = nc.dram_tensor("wk", [C, HD], F32R, kind="ExternalInput")
    wv = nc.dram_tensor("wv", [C, HD], F32R, kind="ExternalInput")
    wo = nc.dram_tensor("wo", [GH * HD, C], F32R, kind="ExternalInput")
    cosT = nc.dram_tensor("cosT", [HD, T], F32, kind="ExternalInput")
    sinT = nc.dram_tensor("sinT", [HD, T], F32, kind="ExternalInput")
    yt = nc.dram_tensor("yt", [C, T], F32, kind="ExternalOutput")

    with tile.TileContext(nc) as tc:
        with (
            tc.tile_pool(name="consts", bufs=1) as consts,
            tc.tile_pool(name="wsmall", bufs=1) as wsmall,
            tc.tile_pool(name="wbig", bufs=1) as wbig,       # Wq then Wo (shared slots)
            tc.tile_pool(name="big8k", bufs=6) as big8k,     # cos,sin then 4x Ot
            tc.tile_pool(name="qk", bufs=1) as qkpool,
            tc.tile_pool(name="xs", bufs=4) as xs,
            tc.tile_pool(name="rope", bufs=3) as ropep,
            tc.tile_pool(name="ptp", bufs=6) as ptp,
            tc.tile_pool(name="pacc", bufs=2) as paccp,
            tc.tile_pool(name="rdp", bufs=2) as rdp,
            tc.tile_pool(name="yo", bufs=3) as yop,
        ):
            # ---- constants (built in f32, converted to f32r via DVE copy) ----
            mbig32 = consts.tile([128, 896], F32)
            nc.gpsimd.memset(mbig32, 1.0)
            nc.gpsimd.affine_select(
                out=mbig32, in_=mbig32,
                compare_op=mybir.AluOpType.is_ge,
                fill=0.0, base=-384,
                pattern=[[1, 896]], channel_multiplier=-1,
            )
            mbig = consts.tile([128, 896], F32R)      # shifted causal masks
            nc.vector.tensor_copy(out=mbig, in_=mbig32)
            ident32 = consts.tile([128, 128], F32)
            make_identity(nc, ident32)
            ident = consts.tile([128, 128], F32R)
            nc.vector.tensor_copy(out=ident, in_=ident32)
            ones32 = consts.tile([128, 1], F32)
            nc.vector.memset(ones32, 1.0)
            ones128 = consts.tile([128, 1], F32R)     # densum lhsT  [K=128, M=1]
            nc.vector.tensor_copy(out=ones128, in_=ones32)
            onesr32 = consts.tile([1, 128], F32)
            nc.vector.memset(onesr32, 1.0)
            ones_row = consts.tile([1, 128], F32R)    # bcast lhsT   [K=1, M=128]
            nc.vector.tensor_copy(out=ones_row, in_=onesr32)

            # ---- resident weights / tables ----
            wq_sb = wbig.tile([128, NCT, GH * HD], F32R, tag="wbig")
            nc.sync.dma_start(out=wq_sb, in_=wq.rearrange("(ct p) n -> p ct n", p=128))
            wk_sb = wsmall.tile([128, NCT, HD], F32R, tag="wk")
            nc.sync.dma_start(out=wk_sb, in_=wk.rearrange("(ct p) n -> p ct n", p=128))
            wv_sb = wsmall.tile([128, NCT, HD], F32R, tag="wv")
            nc.sync.dma_start(out=wv_sb, in_=wv.rearrange("(ct p) n -> p ct n", p=128))
            cos_sb = big8k.tile([128, T], F32, tag="big8k")
            nc.sync.dma_start(out=cos_sb, in_=cosT[:, :])
            sin_sb = big8k.tile([128, T], F32, tag="big8k")
            nc.sync.dma_start(out=sin_sb, in_=sinT[:, :])

            qt_sb = [qkpool.tile([128, T], F32R, tag=f"qt{h}", name=f"qt{h}")
                     for h in range(GH)]
            kt_sb = qkpool.tile([128, T], F32R, tag="kt")
            v_sb = qkpool.tile([128, NT, HD], F32R, tag="v")

            # ================= phase 1: projections =================
            def rope_store(ps, dest, tb):
                """dest[:, tb*512:(tb+1)*512] = rope(ps) ; ps is [128(d), 512(t)]"""
                sl = slice(tb * 512, (tb + 1) * 512)
                a = ropep.tile([128, 512], F32, tag="ropea")
                nc.vector.tensor_mul(a, ps, cos_sb[:, sl])
                b = ropep.tile([128, 512], F32, tag="ropeb")
                nc.vector.tensor_mul(b[0:64], ps[64:128], sin_sb[0:64, sl])
                nc.vector.tensor_mul(b[64:128], ps[0:64], sin_sb[64:128, sl])
                nc.vector.tensor_sub(dest[0:64, sl], a[0:64], b[0:64])
                nc.vector.tensor_add(dest[64:128, sl], a[64:128], b[64:128])

            with (
                tc.tile_pool(name="pp", bufs=6, space="PSUM") as pp,
                tc.tile_pool(name="pvt", bufs=2, space="PSUM") as pvt,
                tc.tile_pool(name="vtt", bufs=2) as vtt,
            ):
                for tb in range(NTB):
                    ps_q = [pp.tile([128, 512], F32, tag="pp", name=f"psq{h}")
                            for h in range(GH)]
                    ps_k = pp.tile([128, 512], F32, tag="pp")
                    ps_v = pp.tile([128, 512], F32, tag="pp")
                    for ct in range(NCT):
                        xt = xs.tile([128, 512], F32R, tag="xs")
                        nc.sync.dma_start(
                            out=xt,
                            in_=xT[ct * 128:(ct + 1) * 128, tb * 512:(tb + 1) * 512],
                        )
                        st, sp = (ct == 0), (ct == NCT - 1)
                        for h in range(GH):
                            nc.tensor.matmul(
                                ps_q[h], (wq_sb[:, ct, h * HD:(h + 1) * HD]),
                                (xt), start=st, stop=sp,
                            )
                        nc.tensor.matmul(
                            ps_k, (wk_sb[:, ct, :]), (xt), start=st, stop=sp)
                        nc.tensor.matmul(
                            ps_v, (wv_sb[:, ct, :]), (xt), start=st, stop=sp)
                    for h in range(GH):
                        rope_store(ps_q[h], qt_sb[h], tb)
                    rope_store(ps_k, kt_sb, tb)
                    # V: copy Vt block to SBUF, PE-transpose each 128x128 tile
                    vt = vtt.tile([128, 512], F32R, tag="vtt")
                    nc.scalar.copy(out=vt, in_=ps_v)
                    for j in range(4):
                        ps_t = pvt.tile([128, 128], F32R, tag="pvt")
                        with nc.allow_low_precision(reason="fp32r PE transpose of V"):
                            nc.tensor.transpose(
                                ps_t, vt[:, j * 128:(j + 1) * 128], ident)
                        nc.scalar.copy(out=v_sb[:, tb * 4 + j, :], in_=ps_t)

            # ================= phase 2: attention =================
            wo_sb = wbig.tile([128, GH, C], F32R, tag="wbig")
            nc.sync.dma_start(out=wo_sb, in_=wo.rearrange("(h p) c -> p h c", p=128))
            ot_sb = [big8k.tile([128, T], F32R, tag="big8k", name=f"ot{h}")
                     for h in range(GH)]

            with (
                tc.tile_pool(name="pst", bufs=4, space="PSUM") as pst,
                tc.tile_pool(name="pot", bufs=2, space="PSUM") as pot,
                tc.tile_pool(name="pd", bufs=1, space="PSUM") as pd,
                tc.tile_pool(name="prdb", bufs=1, space="PSUM") as prdb,
            ):
                for h in range(GH):
                    for qb in range(NTB):
                        nkt = 4 * qb + 4
                        qsl = slice(qb * 512, (qb + 1) * 512)
                        ps_ot = pot.tile([128, 512], F32, tag="pot")
                        pacc = paccp.tile([128, 512], F32R, tag="pacc")
                        pts = [None] * nkt
                        ps_d = None

                        def emit_st(kt):
                            ps_st = pst.tile([128, 512], F32, tag="pst")
                            nc.tensor.matmul(
                                ps_st, (kt_sb[:, kt * 128:(kt + 1) * 128]),
                                (qt_sb[h][:, qsl]), start=True, stop=True,
                            )
                            pt = ptp.tile([128, 512], F32R, tag="pt")
                            nc.scalar.activation(
                                out=pt, in_=ps_st,
                                func=mybir.ActivationFunctionType.Exp, scale=SCALE,
                            )
                            if kt >= 4 * qb:  # diagonal block: causal mask
                                i = kt - 4 * qb
                                nc.vector.tensor_mul(
                                    pt, pt, mbig[:, 384 - 128 * i: 896 - 128 * i])
                            if kt == 0:
                                nc.vector.tensor_copy(out=pacc, in_=pt)
                            else:
                                nc.vector.tensor_add(pacc, pacc, pt)
                            pts[kt] = pt

                        def emit_pv(kt):
                            nc.tensor.matmul(
                                ps_ot, (v_sb[:, kt, :]), (pts[kt]),
                                start=(kt == 0), stop=(kt == nkt - 1),
                            )

                        for kt in range(nkt):
                            emit_st(kt)
                            if kt == nkt - 1:  # denominators (only needs pacc)
                                ps_d = pd.tile([1, 512], F32, tag="pd")
                                nc.tensor.matmul(
                                    ps_d, (ones128), (pacc),
                                    start=True, stop=True)
                            if kt >= PV_PIPE:
                                emit_pv(kt - PV_PIPE)
                        for kt in range(max(0, nkt - PV_PIPE), nkt):
                            emit_pv(kt)

                        rd = rdp.tile([1, 512], F32R, tag="rd")
                        with nc.allow_low_precision(reason="softmax denom recip to fp32r"):
                            nc.vector.reciprocal(out=rd, in_=ps_d)
                        ps_rdb = prdb.tile([128, 512], F32, tag="prdb")
                        nc.tensor.matmul(
                            ps_rdb, (ones_row), (rd), start=True, stop=True)
                        rdb_sb = ropep.tile([128, 512], F32, tag="ropea",
                                            name=f"rdb{h}_{qb}")
                        nc.scalar.copy(out=rdb_sb, in_=ps_rdb)
                        nc.vector.tensor_mul(ot_sb[h][:, qsl], ps_ot, rdb_sb)

            # ================= phase 3: output projection =================
            with tc.tile_pool(name="py", bufs=4, space="PSUM") as py:
                for ct in range(NCT):
                    for tb in range(NTB):
                        ps_y = py.tile([128, 512], F32, tag="py")
                        for h in range(GH):
                            nc.tensor.matmul(
                                ps_y, (wo_sb[:, h, ct * 128:(ct + 1) * 128]),
                                (ot_sb[h][:, tb * 512:(tb + 1) * 512]),
                                start=(h == 0), stop=(h == GH - 1),
                            )
                        yo = yop.tile([128, 512], F32, tag="yo")
                        nc.vector.tensor_copy(out=yo, in_=ps_y)
                        nc.sync.dma_start(
                            out=yt[ct * 128:(ct + 1) * 128, tb * 512:(tb + 1) * 512],
                            in_=yo,
                        )
    _split_multi_waits(nc)
    return nc


def _rope_tables():
    inv_freq = 1.0 / (ROPE_THETA ** (np.arange(0, HD, 2, dtype=np.float32) / HD))
    t = np.arange(T, dtype=np.float32)
    freqs = np.outer(t, inv_freq)                    # [T, HD/2]
    emb = np.concatenate([freqs, freqs], axis=-1)    # [T, HD]
    cosT = np.ascontiguousarray(np.cos(emb).T.astype(np.float32))  # [HD, T]
    sinT = np.ascontiguousarray(np.sin(emb).T.astype(np.float32))
    return cosT, sinT


_NC_CACHE = {}


def kernel(x, Wq, Wk, Wv, Wo, _trace=False, _trace_kwargs=None):
    x = np.asarray(x, np.float32)
    Wq, Wk, Wv, Wo = (np.asarray(w, np.float32) for w in (Wq, Wk, Wv, Wo))
    if "nc" not in _NC_CACHE:
        _NC_CACHE["nc"] = build_kernel()
        _NC_CACHE["rope"] = _rope_tables()
    nc = _NC_CACHE["nc"]

    cosT, sinT = _NC_CACHE["rope"]
    in_maps = []
    for b in range(B):
        xTb = np.ascontiguousarray(x[b].T)
        for g in range(NKV):
            in_maps.append({
                "xT": xTb,
                "wq": np.ascontiguousarray(Wq[:, g * 512:(g + 1) * 512]),
                "wk": np.ascontiguousarray(Wk[:, g * 128:(g + 1) * 128]),
                "wv": np.ascontiguousarray(Wv[:, g * 128:(g + 1) * 128]),
                "wo": np.ascontiguousarray(Wo[g * 512:(g + 1) * 512, :]),
                "cosT": cosT,
                "sinT": sinT,
            })

    kwargs = {}
    if _trace:
        kwargs["trace"] = True
        kwargs.update(_trace_kwargs or {})
    res = run_bass_kernel_spmd(nc, in_maps, core_ids=list(range(8)), **kwargs)

    y = np.empty((B, T, C), np.float32)
    for b in range(B):
        acc = res.results[b * NKV]["yt"] + res.results[b * NKV + 1]["yt"]
        for g in range(2, NKV):
            acc += res.results[b * NKV + g]["yt"]
        y[b] = acc.T
    if _trace:
        return y, res
    return y
